# revision 10
# baseline (speedup 1.0000x reference)
"""MoE model kernel for Trainium2 (8 NeuronCores, data-parallel over batch).

Reference computation (per token):
  router: 3-layer MLP (fp32) -> softmax -> top-2 gates (vals/2 scattered dense)
  experts: 8x (D->H1 relu, H1->H2 relu, H2->C) combined with gates
Outputs: (out [B, C] f32, probs [B, E] f32)

Sharding: batch B=16384 split across 8 cores (2048 tokens each); router and
all experts replicated on every core. Router runs in fp32 on the PE (top-2
selection needs fp32 accuracy: min prob gap between ranks 2/3 is ~1.5e-6).
Expert matmuls run in bf16 with fp32 PSUM accumulation (max err ~0.3% of
output scale). The dense gated combine matches the reference's math exactly
(gates are 0 for non-selected experts).
"""

import numpy as np
import ml_dtypes

import concourse.bacc as bacc
import concourse.bass as bass
import concourse.mybir as mybir
import concourse.tile as tile
from concourse.bass_utils import run_bass_kernel_spmd

F32 = mybir.dt.float32
BF16 = mybir.dt.bfloat16
AF = mybir.ActivationFunctionType
ALU = mybir.AluOpType
AX = mybir.AxisListType

B, D, E, C = 16384, 1024, 8, 50
RH = 512
H1, H2 = 2048, 1024
NCORES = 8
T = B // NCORES  # tokens per core
P = 128
NT = T // P      # 16 token tiles per core
CP = 64          # padded C for psum/acc tiles

_built = None


def _router(nc, tc, io, pools):
    """Router MLP in fp32 + softmax + top-2 gates. Writes probs to DRAM.

    Returns the gates tile [128, NT, E] f32 (token-major) in a long-lived pool.
    """
    gpool = pools["gates"]
    gates = gpool.tile([P, NT, E], F32, tag="gates", bufs=1)
    probs_sb = gpool.tile([P, NT, E], F32, tag="probs", bufs=1)

    with (
        tc.tile_pool(name="rsb", bufs=1) as rsb,
        tc.tile_pool(name="rstream", bufs=3) as rstream,
        tc.tile_pool(name="rps", bufs=1, space="PSUM") as rps,
    ):
        # Router weights resident in SBUF (fp32, ~2.6MB total)
        rW1_sb = rsb.tile([P, D // P, RH], F32, tag="rW1")
        nc.sync.dma_start(rW1_sb[:], io["rW1"][:].rearrange("(k p) m -> p k m", p=P))
        rW2_sb = rsb.tile([P, RH // P, RH // 2], F32, tag="rW2")
        nc.sync.dma_start(rW2_sb[:], io["rW2"][:].rearrange("(k p) m -> p k m", p=P))
        rW3_sb = rsb.tile([P, (RH // 2) // P, E], F32, tag="rW3")
        nc.sync.dma_start(rW3_sb[:], io["rW3"][:].rearrange("(k p) m -> p k m", p=P))
        rb1_sb = rsb.tile([P, RH // P], F32, tag="rb1")
        nc.sync.dma_start(rb1_sb[:], io["rb1"][:].rearrange("(m p) -> p m", p=P))
        rb2_sb = rsb.tile([P, (RH // 2) // P], F32, tag="rb2")
        nc.sync.dma_start(rb2_sb[:], io["rb2"][:].rearrange("(m p) -> p m", p=P))
        rb3_sb = rsb.tile([1, E], F32, tag="rb3")
        nc.sync.dma_start(rb3_sb[:], io["rb3"][None, :])
        ones1 = rsb.tile([1, P], F32, tag="ones1")
        nc.vector.memset(ones1[:], 1.0)

        h1r = rsb.tile([P, RH // P, T], F32, tag="h1r")   # 4MB
        h2r = rsb.tile([P, (RH // 2) // P, T], F32, tag="h2r")  # 2MB

        NTOK = 512  # token tile for router layers
        xT_view = io["xT_f32"][:].rearrange("(k p) t -> p k t", p=P)

        # L1: h1r = relu(rW1.T @ xT + rb1)
        for n in range(T // NTOK):
            nsl = bass.ts(n, NTOK)
            xks = []
            for k in range(D // P):
                xk = rstream.tile([P, NTOK], F32, tag=f"xk{k % 2}", bufs=2)
                nc.sync.dma_start(xk[:], xT_view[:, k, nsl])
                xks.append(xk)
            psl1 = [
                rps.tile([P, NTOK], F32, tag=f"psl1_{m}", bufs=1, name=f"psl1_{m}")
                for m in range(RH // P)
            ]
            for k in range(D // P):
                for m in range(RH // P):
                    nc.tensor.matmul(
                        psl1[m][:],
                        rW1_sb[:, k, bass.ts(m, P)],
                        xks[k][:],
                        start=(k == 0),
                        stop=(k == D // P - 1),
                    )
            for m in range(RH // P):
                nc.scalar.activation(
                    h1r[:, m, nsl], psl1[m][:], AF.Relu, bias=rb1_sb[:, m : m + 1]
                )

        # L2: h2r = relu(rW2.T @ h1r + rb2)
        for n in range(T // NTOK):
            nsl = bass.ts(n, NTOK)
            psl2 = [
                rps.tile([P, NTOK], F32, tag=f"psl2_{m}", bufs=1, name=f"psl2_{m}")
                for m in range((RH // 2) // P)
            ]
            for k in range(RH // P):
                for m in range((RH // 2) // P):
                    nc.tensor.matmul(
                        psl2[m][:],
                        rW2_sb[:, k, bass.ts(m, P)],
                        h1r[:, k, nsl],
                        start=(k == 0),
                        stop=(k == RH // P - 1),
                    )
            for m in range((RH // 2) // P):
                nc.scalar.activation(
                    h2r[:, m, nsl], psl2[m][:], AF.Relu, bias=rb2_sb[:, m : m + 1]
                )

        # L3 (token-major): scores[t, e] = h2r.T @ rW3 + rb3
        scores = gpool.tile([P, NT, E], F32, tag="scores", bufs=1)
        for i in range(NT):
            ps3 = rps.tile([P, E], F32, tag="ps3", bufs=2)
            for k in range((RH // 2) // P):
                nc.tensor.matmul(
                    ps3[:],
                    h2r[:, k, bass.ts(i, P)],
                    rW3_sb[:, k, :],
                    start=(k == 0),
                    stop=False,
                )
            nc.tensor.matmul(ps3[:], ones1[:], rb3_sb[:], start=False, stop=True)
            nc.vector.tensor_copy(scores[:, i, :], ps3[:])

        # Softmax over E (per 8-wide segment)
        mx = rsb.tile([P, NT, 1], F32, tag="mx")
        nc.vector.tensor_reduce(mx[:], scores[:], axis=AX.X, op=ALU.max)
        xs = rsb.tile([P, NT, E], F32, tag="xs")
        nc.vector.tensor_tensor(
            xs[:], scores[:], mx[:].to_broadcast([P, NT, E]), op=ALU.subtract
        )
        ex = rsb.tile([P, NT, E], F32, tag="ex")
        nc.scalar.activation(ex[:], xs[:], AF.Exp)
        sm = rsb.tile([P, NT, 1], F32, tag="sm")
        nc.vector.tensor_reduce(sm[:], ex[:], axis=AX.X, op=ALU.add)
        rs = rsb.tile([P, NT, 1], F32, tag="rs")
        nc.vector.reciprocal(rs[:], sm[:])
        nc.vector.tensor_tensor(
            probs_sb[:], ex[:], rs[:].to_broadcast([P, NT, E]), op=ALU.mult
        )
        nc.sync.dma_start(
            io["probs"][:].rearrange("(i p) e -> p i e", p=P), probs_sb[:]
        )

        # Top-2 gates: gates = probs * (probs >= 2nd_max) / 2
        m1 = rsb.tile([P, NT, 1], F32, tag="m1")
        nc.vector.tensor_reduce(m1[:], probs_sb[:], axis=AX.X, op=ALU.max)
        lt = rsb.tile([P, NT, E], F32, tag="lt")
        nc.vector.tensor_tensor(
            lt[:], probs_sb[:], m1[:].to_broadcast([P, NT, E]), op=ALU.is_lt
        )
        pz = rsb.tile([P, NT, E], F32, tag="pz")
        nc.vector.tensor_tensor(pz[:], probs_sb[:], lt[:], op=ALU.mult)
        m2 = rsb.tile([P, NT, 1], F32, tag="m2")
        nc.vector.tensor_reduce(m2[:], pz[:], axis=AX.X, op=ALU.max)
        ge = rsb.tile([P, NT, E], F32, tag="ge")
        nc.vector.tensor_tensor(
            ge[:], probs_sb[:], m2[:].to_broadcast([P, NT, E]), op=ALU.is_ge
        )
        ph = rsb.tile([P, NT, E], F32, tag="ph")
        nc.vector.tensor_scalar_mul(ph[:], probs_sb[:], 0.5)
        nc.vector.tensor_tensor(gates[:], ph[:], ge[:], op=ALU.mult)

    return gates


def _experts(nc, tc, io, pools, gates):
    """Dense expert compute in bf16 with gated fp32 combine."""
    gpool = pools["gates"]
    acc = gpool.tile([P, NT, CP], F32, tag="acc", bufs=1)
    nc.vector.memset(acc[:], 0.0)

    TH = 1024       # tokens per half
    NTOK = 512      # matmul free dim
    NH = T // TH    # 2 halves

    with (
        tc.tile_pool(name="esb", bufs=1) as esb,
        tc.tile_pool(name="ew", bufs=1) as ew,
        tc.tile_pool(name="eps", bufs=1, space="PSUM") as eps,
    ):
        xTb_sb = esb.tile([P, D // P, T], BF16, tag="xTb")  # 4MB resident
        nc.sync.dma_start(
            xTb_sb[:], io["xT_bf16"][:].rearrange("(k p) t -> p k t", p=P)
        )
        onesb = esb.tile([1, P], BF16, tag="onesb")
        nc.vector.memset(onesb[:], 1.0)
        h1b = esb.tile([P, H1 // P, TH], BF16, tag="h1b")  # 4MB
        h2b = esb.tile([P, H2 // P, TH], BF16, tag="h2b")  # 2MB

        for e in range(E):
            eb1_sb = ew.tile([P, H1 // P], F32, tag="eb1", bufs=2)
            nc.sync.dma_start(
                eb1_sb[:], io[f"eb1_{e}"][:].rearrange("(m p) -> p m", p=P)
            )
            eb2_sb = ew.tile([P, H2 // P], F32, tag="eb2", bufs=2)
            nc.sync.dma_start(
                eb2_sb[:], io[f"eb2_{e}"][:].rearrange("(m p) -> p m", p=P)
            )
            w3_sb = ew.tile([P, H2 // P, C], BF16, tag="w3", bufs=2)
            nc.sync.dma_start(
                w3_sb[:], io[f"eW3_{e}"][:].rearrange("(k p) m -> p k m", p=P)
            )
            b3_sb = ew.tile([1, C], BF16, tag="b3", bufs=2)
            nc.sync.dma_start(b3_sb[:], io[f"eb3b_{e}"][:])

            w1_view = io[f"eW1_{e}"][:].rearrange("(k p) m -> p k m", p=P)
            w2_view = io[f"eW2_{e}"][:].rearrange("(k p) m -> p k m", p=P)

            for h in range(NH):
                hsl = slice(h * TH, (h + 1) * TH)
                # L1: h1b = relu(W1.T @ x + b1), M=H1 in 4 strips of 512
                for mi in range(H1 // NTOK):
                    w1s = ew.tile([P, D // P, NTOK], BF16, tag="w1s", bufs=3)
                    nc.sync.dma_start(w1s[:], w1_view[:, :, bass.ts(mi, NTOK)])
                    for mm in range(NTOK // P):
                        m = mi * (NTOK // P) + mm
                        for n in range(TH // NTOK):
                            nsl = slice(h * TH + n * NTOK, h * TH + (n + 1) * NTOK)
                            ps = eps.tile([P, NTOK], F32, tag="l1ps", bufs=3)
                            for k in range(D // P):
                                nc.tensor.matmul(
                                    ps[:],
                                    w1s[:, k, bass.ts(mm, P)],
                                    xTb_sb[:, k, nsl],
                                    start=(k == 0),
                                    stop=(k == D // P - 1),
                                )
                            nc.scalar.activation(
                                h1b[:, m, bass.ts(n, NTOK)],
                                ps[:],
                                AF.Relu,
                                bias=eb1_sb[:, m : m + 1],
                            )
                # L2: h2b = relu(W2.T @ h1b + b2), M=H2 in 2 strips of 512
                for mi in range(H2 // NTOK):
                    w2s = ew.tile([P, H1 // P, NTOK], BF16, tag="w2s", bufs=2)
                    nc.sync.dma_start(w2s[:], w2_view[:, :, bass.ts(mi, NTOK)])
                    for mm in range(NTOK // P):
                        m = mi * (NTOK // P) + mm
                        for n in range(TH // NTOK):
                            ps = eps.tile([P, NTOK], F32, tag="l2ps", bufs=3)
                            for k in range(H1 // P):
                                nc.tensor.matmul(
                                    ps[:],
                                    w2s[:, k, bass.ts(mm, P)],
                                    h1b[:, k, bass.ts(n, NTOK)],
                                    start=(k == 0),
                                    stop=(k == H1 // P - 1),
                                )
                            nc.scalar.activation(
                                h2b[:, m, bass.ts(n, NTOK)],
                                ps[:],
                                AF.Relu,
                                bias=eb2_sb[:, m : m + 1],
                            )
                # L3 token-major + gated combine
                for i in range(TH // P):
                    it = h * (TH // P) + i
                    ps3 = eps.tile([P, CP], F32, tag="l3ps", bufs=2)
                    for k in range(H2 // P):
                        nc.tensor.matmul(
                            ps3[:, :C],
                            h2b[:, k, bass.ts(i, P)],
                            w3_sb[:, k, :],
                            start=(k == 0),
                            stop=False,
                        )
                    nc.tensor.matmul(
                        ps3[:, :C], onesb[:], b3_sb[:], start=False, stop=True
                    )
                    tmp = ew.tile([P, CP], F32, tag="tmp", bufs=4)
                    nc.scalar.activation(
                        tmp[:, :C], ps3[:, :C], AF.Copy,
                        scale=gates[:, it, e : e + 1],
                    )
                    nc.vector.tensor_add(
                        acc[:, it, :C], acc[:, it, :C], tmp[:, :C]
                    )

        nc.sync.dma_start(io["out"][:].rearrange("(i p) c -> p i c", p=P), acc[:, :, :C])


def build():
    nc = bacc.Bacc(None, target_bir_lowering=False, debug=False)

    io = {}
    io["xT_f32"] = nc.dram_tensor("xT_f32", [D, T], F32, kind="ExternalInput")
    io["xT_bf16"] = nc.dram_tensor("xT_bf16", [D, T], BF16, kind="ExternalInput")
    io["rW1"] = nc.dram_tensor("rW1", [D, RH], F32, kind="ExternalInput")
    io["rb1"] = nc.dram_tensor("rb1", [RH], F32, kind="ExternalInput")
    io["rW2"] = nc.dram_tensor("rW2", [RH, RH // 2], F32, kind="ExternalInput")
    io["rb2"] = nc.dram_tensor("rb2", [RH // 2], F32, kind="ExternalInput")
    io["rW3"] = nc.dram_tensor("rW3", [RH // 2, E], F32, kind="ExternalInput")
    io["rb3"] = nc.dram_tensor("rb3", [E], F32, kind="ExternalInput")
    for e in range(E):
        io[f"eW1_{e}"] = nc.dram_tensor(f"eW1_{e}", [D, H1], BF16, kind="ExternalInput")
        io[f"eb1_{e}"] = nc.dram_tensor(f"eb1_{e}", [H1], F32, kind="ExternalInput")
        io[f"eW2_{e}"] = nc.dram_tensor(f"eW2_{e}", [H1, H2], BF16, kind="ExternalInput")
        io[f"eb2_{e}"] = nc.dram_tensor(f"eb2_{e}", [H2], F32, kind="ExternalInput")
        io[f"eW3_{e}"] = nc.dram_tensor(f"eW3_{e}", [H2, C], BF16, kind="ExternalInput")
        io[f"eb3b_{e}"] = nc.dram_tensor(f"eb3b_{e}", [1, C], BF16, kind="ExternalInput")
    io["out"] = nc.dram_tensor("out", [T, C], F32, kind="ExternalOutput")
    io["probs"] = nc.dram_tensor("probs", [T, E], F32, kind="ExternalOutput")

    with tile.TileContext(nc) as tc:
        with tc.tile_pool(name="gates_pool", bufs=1) as gpool:
            pools = {"gates": gpool}
            gates = _router(nc, tc, io, pools)
            _experts(nc, tc, io, pools, gates)

    nc.compile()
    return nc


def _get_built():
    global _built
    if _built is None:
        _built = build()
    return _built


def make_in_maps(inputs):
    """Shard FULL inputs into per-core in_maps."""
    x = np.asarray(inputs["x"], np.float32)
    eW1 = np.asarray(inputs["eW1"], np.float32)
    eW2 = np.asarray(inputs["eW2"], np.float32)
    eW3 = np.asarray(inputs["eW3"], np.float32)
    eb1 = np.asarray(inputs["eb1"], np.float32)
    eb2 = np.asarray(inputs["eb2"], np.float32)
    eb3 = np.asarray(inputs["eb3"], np.float32)
    shared = {}
    for k in ("rW1", "rb1", "rW2", "rb2", "rW3", "rb3"):
        shared[k] = np.ascontiguousarray(np.asarray(inputs[k], np.float32))
    for e in range(E):
        shared[f"eW1_{e}"] = np.ascontiguousarray(eW1[e].astype(ml_dtypes.bfloat16))
        shared[f"eW2_{e}"] = np.ascontiguousarray(eW2[e].astype(ml_dtypes.bfloat16))
        shared[f"eW3_{e}"] = np.ascontiguousarray(eW3[e].astype(ml_dtypes.bfloat16))
        shared[f"eb1_{e}"] = np.ascontiguousarray(eb1[e])
        shared[f"eb2_{e}"] = np.ascontiguousarray(eb2[e])
        shared[f"eb3b_{e}"] = np.ascontiguousarray(
            eb3[e].astype(ml_dtypes.bfloat16)[None, :]
        )
    in_maps = []
    for c in range(NCORES):
        xs = x[c * T : (c + 1) * T]
        xT = np.ascontiguousarray(xs.T)
        m = dict(shared)
        m["xT_f32"] = xT
        m["xT_bf16"] = xT.astype(ml_dtypes.bfloat16)
        in_maps.append(m)
    return in_maps


def kernel(**inputs):
    assert int(inputs.get("top_k", 2)) == 2
    nc = _get_built()
    in_maps = make_in_maps(inputs)
    res = run_bass_kernel_spmd(nc, in_maps, core_ids=list(range(NCORES)))
    out = np.concatenate([res.results[c]["out"] for c in range(NCORES)], axis=0)
    probs = np.concatenate([res.results[c]["probs"] for c in range(NCORES)], axis=0)
    return out, probs


# revision 17
# speedup vs baseline: 78.2502x; 78.2502x over previous
"""MoE model kernel for Trainium2 (8 NeuronCores, data-parallel over batch).

Reference computation (per token):
  router: 3-layer MLP (fp32) -> softmax -> top-2 gates (vals/2 scattered dense)
  experts: 8x (D->H1 relu, H1->H2 relu, H2->C) combined with gates
Outputs: (out [B, C] f32, probs [B, E] f32)

Sharding: batch B=16384 split across 8 cores (2048 tokens each); router and
all experts replicated on every core. Router runs in fp32 on the PE (top-2
selection needs fp32 accuracy: min prob gap between ranks 2/3 is ~1.5e-6).
Expert matmuls run in bf16 with fp32 PSUM accumulation (max err ~0.3% of
output scale). The dense gated combine matches the reference's math exactly
(gates are 0 for non-selected experts).
"""

import numpy as np
import ml_dtypes

import concourse.bacc as bacc
import concourse.bass as bass
import concourse.mybir as mybir
import concourse.tile as tile
from concourse.bass_utils import run_bass_kernel_spmd

F32 = mybir.dt.float32
BF16 = mybir.dt.bfloat16
AF = mybir.ActivationFunctionType
ALU = mybir.AluOpType
AX = mybir.AxisListType

B, D, E, C = 16384, 1024, 8, 50
RH = 512
H1, H2 = 2048, 1024
NCORES = 8
T = B // NCORES  # tokens per core
P = 128
NT = T // P      # 16 token tiles per core
CP = 64          # padded C for psum/acc tiles

# Sparse dispatch: per-expert slot capacities (multiples of 128). Sized from
# the routing distribution of this model's router (counts per 2048-token
# shard range ~115..1240 across experts) with >5 sigma margin. Tokens beyond
# capacity would be dropped, so test.py asserts actual counts fit.
CAPS = (640, 1536, 256, 256, 768, 1152, 1152, 384)
OFFS = tuple(int(np.sum(CAPS[:e])) for e in range(E))
NSLOT = int(np.sum(CAPS))          # 6144
NPAD = T + P                        # outpad rows (last 128 = trash tokens)
CHUNK = 768                         # max slots per compute chunk

VARIANT = "sparse"

_built = None


def _router(nc, tc, io, pools):
    """Router MLP in fp32 + softmax + top-2 gates.

    Writes probs into acc[:, :, 50:58] (packed output). Returns the gates
    tile [128, NT, E] f32 (token-major) in a long-lived pool.
    """
    gpool = pools["gates"]
    acc = pools["acc"]
    gates = gpool.tile([P, NT, E], F32, tag="gates", bufs=1)
    probs_sb = gpool.tile([P, NT, E], F32, tag="probs", bufs=1)

    with (
        tc.tile_pool(name="rsb", bufs=1) as rsb,
        tc.tile_pool(name="rstream", bufs=3) as rstream,
        tc.tile_pool(name="rps", bufs=1, space="PSUM") as rps,
    ):
        # Router weights resident in SBUF (fp32, ~2.6MB total)
        rW1_sb = rsb.tile([P, D // P, RH], F32, tag="rW1")
        nc.sync.dma_start(rW1_sb[:], io["rW1"][:].rearrange("(k p) m -> p k m", p=P))
        rW2_sb = rsb.tile([P, RH // P, RH // 2], F32, tag="rW2")
        nc.sync.dma_start(rW2_sb[:], io["rW2"][:].rearrange("(k p) m -> p k m", p=P))
        rW3_sb = rsb.tile([P, (RH // 2) // P, E], F32, tag="rW3")
        nc.sync.dma_start(rW3_sb[:], io["rW3"][:].rearrange("(k p) m -> p k m", p=P))
        rb1_sb = rsb.tile([P, RH // P], F32, tag="rb1")
        nc.sync.dma_start(rb1_sb[:], io["rb1"][:].rearrange("(m p) -> p m", p=P))
        rb2_sb = rsb.tile([P, (RH // 2) // P], F32, tag="rb2")
        nc.sync.dma_start(rb2_sb[:], io["rb2"][:].rearrange("(m p) -> p m", p=P))
        rb3_sb = rsb.tile([1, E], F32, tag="rb3")
        nc.sync.dma_start(rb3_sb[:], io["rb3"][None, :])
        ones1 = rsb.tile([1, P], F32, tag="ones1")
        nc.vector.memset(ones1[:], 1.0)

        h1r = rsb.tile([P, RH // P, T], F32, tag="h1r")   # 4MB
        h2r = rsb.tile([P, (RH // 2) // P, T], F32, tag="h2r")  # 2MB

        NTOK = 512  # token tile for router layers
        xT_view = io["xT_f32"][:].rearrange("(k p) t -> p k t", p=P)

        # L1: h1r = relu(rW1.T @ xT + rb1)
        for n in range(T // NTOK):
            nsl = bass.ts(n, NTOK)
            xks = []
            for k in range(D // P):
                xk = rstream.tile([P, NTOK], F32, tag=f"xk{k % 2}", bufs=2)
                nc.sync.dma_start(xk[:], xT_view[:, k, nsl])
                xks.append(xk)
            psl1 = [
                rps.tile([P, NTOK], F32, tag=f"psl1_{m}", bufs=1, name=f"psl1_{m}")
                for m in range(RH // P)
            ]
            for k in range(D // P):
                for m in range(RH // P):
                    nc.tensor.matmul(
                        psl1[m][:],
                        rW1_sb[:, k, bass.ts(m, P)],
                        xks[k][:],
                        start=(k == 0),
                        stop=(k == D // P - 1),
                    )
            for m in range(RH // P):
                nc.scalar.activation(
                    h1r[:, m, nsl], psl1[m][:], AF.Relu, bias=rb1_sb[:, m : m + 1]
                )

        # L2: h2r = relu(rW2.T @ h1r + rb2)
        for n in range(T // NTOK):
            nsl = bass.ts(n, NTOK)
            psl2 = [
                rps.tile([P, NTOK], F32, tag=f"psl2_{m}", bufs=1, name=f"psl2_{m}")
                for m in range((RH // 2) // P)
            ]
            for k in range(RH // P):
                for m in range((RH // 2) // P):
                    nc.tensor.matmul(
                        psl2[m][:],
                        rW2_sb[:, k, bass.ts(m, P)],
                        h1r[:, k, nsl],
                        start=(k == 0),
                        stop=(k == RH // P - 1),
                    )
            for m in range((RH // 2) // P):
                nc.scalar.activation(
                    h2r[:, m, nsl], psl2[m][:], AF.Relu, bias=rb2_sb[:, m : m + 1]
                )

        # L3 (token-major): scores[t, e] = h2r.T @ rW3 + rb3
        scores = gpool.tile([P, NT, E], F32, tag="scores", bufs=1)
        for i in range(NT):
            ps3 = rps.tile([P, E], F32, tag="ps3", bufs=2)
            for k in range((RH // 2) // P):
                nc.tensor.matmul(
                    ps3[:],
                    h2r[:, k, bass.ts(i, P)],
                    rW3_sb[:, k, :],
                    start=(k == 0),
                    stop=False,
                )
            nc.tensor.matmul(ps3[:], ones1[:], rb3_sb[:], start=False, stop=True)
            nc.vector.tensor_copy(scores[:, i, :], ps3[:])

        # Softmax over E (per 8-wide segment)
        mx = rsb.tile([P, NT, 1], F32, tag="mx")
        nc.vector.tensor_reduce(mx[:], scores[:], axis=AX.X, op=ALU.max)
        xs = rsb.tile([P, NT, E], F32, tag="xs")
        nc.vector.tensor_tensor(
            xs[:], scores[:], mx[:].to_broadcast([P, NT, E]), op=ALU.subtract
        )
        ex = rsb.tile([P, NT, E], F32, tag="ex")
        nc.scalar.activation(ex[:], xs[:], AF.Exp)
        sm = rsb.tile([P, NT, 1], F32, tag="sm")
        nc.vector.tensor_reduce(sm[:], ex[:], axis=AX.X, op=ALU.add)
        rs = rsb.tile([P, NT, 1], F32, tag="rs")
        nc.vector.reciprocal(rs[:], sm[:])
        nc.vector.tensor_tensor(
            probs_sb[:], ex[:], rs[:].to_broadcast([P, NT, E]), op=ALU.mult
        )
        nc.vector.tensor_copy(acc[:, :, C : C + E], probs_sb[:])

        # Top-2 gates: gates = probs * (probs >= 2nd_max) / 2
        m1 = rsb.tile([P, NT, 1], F32, tag="m1")
        nc.vector.tensor_reduce(m1[:], probs_sb[:], axis=AX.X, op=ALU.max)
        lt = rsb.tile([P, NT, E], F32, tag="lt")
        nc.vector.tensor_tensor(
            lt[:], probs_sb[:], m1[:].to_broadcast([P, NT, E]), op=ALU.is_lt
        )
        pz = rsb.tile([P, NT, E], F32, tag="pz")
        nc.vector.tensor_tensor(pz[:], probs_sb[:], lt[:], op=ALU.mult)
        m2 = rsb.tile([P, NT, 1], F32, tag="m2")
        nc.vector.tensor_reduce(m2[:], pz[:], axis=AX.X, op=ALU.max)
        ge = rsb.tile([P, NT, E], F32, tag="ge")
        nc.vector.tensor_tensor(
            ge[:], probs_sb[:], m2[:].to_broadcast([P, NT, E]), op=ALU.is_ge
        )
        ph = rsb.tile([P, NT, E], F32, tag="ph")
        nc.vector.tensor_scalar_mul(ph[:], probs_sb[:], 0.5)
        nc.vector.tensor_tensor(gates[:], ph[:], ge[:], op=ALU.mult)

    return gates


def _experts(nc, tc, io, pools, gates):
    """Dense expert compute in bf16 with gated fp32 combine."""
    acc = pools["acc"]

    TH = 1024       # tokens per half
    NTOK = 512      # matmul free dim
    NH = T // TH    # 2 halves

    with (
        tc.tile_pool(name="esb", bufs=1) as esb,
        tc.tile_pool(name="ew", bufs=1) as ew,
        tc.tile_pool(name="eps", bufs=1, space="PSUM") as eps,
    ):
        xTb_sb = esb.tile([P, D // P, T], BF16, tag="xTb")  # 4MB resident
        nc.sync.dma_start(
            xTb_sb[:], io["xT_bf16"][:].rearrange("(k p) t -> p k t", p=P)
        )
        onesb = esb.tile([1, P], BF16, tag="onesb")
        nc.vector.memset(onesb[:], 1.0)
        h1b = esb.tile([P, H1 // P, TH], BF16, tag="h1b")  # 4MB
        h2b = esb.tile([P, H2 // P, TH], BF16, tag="h2b")  # 2MB

        for e in range(E):
            eb1_sb = ew.tile([P, H1 // P], F32, tag="eb1", bufs=2)
            nc.sync.dma_start(
                eb1_sb[:], io[f"eb1_{e}"][:].rearrange("(m p) -> p m", p=P)
            )
            eb2_sb = ew.tile([P, H2 // P], F32, tag="eb2", bufs=2)
            nc.sync.dma_start(
                eb2_sb[:], io[f"eb2_{e}"][:].rearrange("(m p) -> p m", p=P)
            )
            w3_sb = ew.tile([P, H2 // P, C], BF16, tag="w3", bufs=2)
            nc.sync.dma_start(
                w3_sb[:], io[f"eW3_{e}"][:].rearrange("(k p) m -> p k m", p=P)
            )
            b3_sb = ew.tile([1, C], BF16, tag="b3", bufs=2)
            nc.sync.dma_start(b3_sb[:], io[f"eb3b_{e}"][:])

            w1_view = io[f"eW1_{e}"][:].rearrange("(k p) m -> p k m", p=P)
            w2_view = io[f"eW2_{e}"][:].rearrange("(k p) m -> p k m", p=P)

            for h in range(NH):
                hsl = slice(h * TH, (h + 1) * TH)
                # L1: h1b = relu(W1.T @ x + b1), M=H1 in 4 strips of 512
                for mi in range(H1 // NTOK):
                    w1s = ew.tile([P, D // P, NTOK], BF16, tag="w1s", bufs=3)
                    nc.sync.dma_start(w1s[:], w1_view[:, :, bass.ts(mi, NTOK)])
                    for mm in range(NTOK // P):
                        m = mi * (NTOK // P) + mm
                        for n in range(TH // NTOK):
                            nsl = slice(h * TH + n * NTOK, h * TH + (n + 1) * NTOK)
                            ps = eps.tile([P, NTOK], F32, tag="l1ps", bufs=3)
                            for k in range(D // P):
                                nc.tensor.matmul(
                                    ps[:],
                                    w1s[:, k, bass.ts(mm, P)],
                                    xTb_sb[:, k, nsl],
                                    start=(k == 0),
                                    stop=(k == D // P - 1),
                                )
                            nc.scalar.activation(
                                h1b[:, m, bass.ts(n, NTOK)],
                                ps[:],
                                AF.Relu,
                                bias=eb1_sb[:, m : m + 1],
                            )
                # L2: h2b = relu(W2.T @ h1b + b2), M=H2 in 2 strips of 512
                for mi in range(H2 // NTOK):
                    w2s = ew.tile([P, H1 // P, NTOK], BF16, tag="w2s", bufs=2)
                    nc.sync.dma_start(w2s[:], w2_view[:, :, bass.ts(mi, NTOK)])
                    for mm in range(NTOK // P):
                        m = mi * (NTOK // P) + mm
                        for n in range(TH // NTOK):
                            ps = eps.tile([P, NTOK], F32, tag="l2ps", bufs=3)
                            for k in range(H1 // P):
                                nc.tensor.matmul(
                                    ps[:],
                                    w2s[:, k, bass.ts(mm, P)],
                                    h1b[:, k, bass.ts(n, NTOK)],
                                    start=(k == 0),
                                    stop=(k == H1 // P - 1),
                                )
                            nc.scalar.activation(
                                h2b[:, m, bass.ts(n, NTOK)],
                                ps[:],
                                AF.Relu,
                                bias=eb2_sb[:, m : m + 1],
                            )
                # L3 token-major + gated combine
                for i in range(TH // P):
                    it = h * (TH // P) + i
                    ps3 = eps.tile([P, CP], F32, tag="l3ps", bufs=2)
                    for k in range(H2 // P):
                        nc.tensor.matmul(
                            ps3[:, :C],
                            h2b[:, k, bass.ts(i, P)],
                            w3_sb[:, k, :],
                            start=(k == 0),
                            stop=False,
                        )
                    nc.tensor.matmul(
                        ps3[:, :C], onesb[:], b3_sb[:], start=False, stop=True
                    )
                    tmp = ew.tile([P, CP], F32, tag="tmp", bufs=4)
                    nc.scalar.activation(
                        tmp[:, :C], ps3[:, :C], AF.Copy,
                        scale=gates[:, it, e : e + 1],
                    )
                    nc.vector.tensor_add(
                        acc[:, it, :C], acc[:, it, :C], tmp[:, :C]
                    )

        nc.sync.dma_start(io["outbuf"][:].rearrange("(i p) c -> p i c", p=P), acc[:])


def build():
    nc = bacc.Bacc(None, target_bir_lowering=False, debug=False)

    io = {}
    io["xT_f32"] = nc.dram_tensor("xT_f32", [D, T], F32, kind="ExternalInput")
    io["xT_bf16"] = nc.dram_tensor("xT_bf16", [D, T], BF16, kind="ExternalInput")
    io["rW1"] = nc.dram_tensor("rW1", [D, RH], F32, kind="ExternalInput")
    io["rb1"] = nc.dram_tensor("rb1", [RH], F32, kind="ExternalInput")
    io["rW2"] = nc.dram_tensor("rW2", [RH, RH // 2], F32, kind="ExternalInput")
    io["rb2"] = nc.dram_tensor("rb2", [RH // 2], F32, kind="ExternalInput")
    io["rW3"] = nc.dram_tensor("rW3", [RH // 2, E], F32, kind="ExternalInput")
    io["rb3"] = nc.dram_tensor("rb3", [E], F32, kind="ExternalInput")
    for e in range(E):
        io[f"eW1_{e}"] = nc.dram_tensor(f"eW1_{e}", [D, H1], BF16, kind="ExternalInput")
        io[f"eb1_{e}"] = nc.dram_tensor(f"eb1_{e}", [H1], F32, kind="ExternalInput")
        io[f"eW2_{e}"] = nc.dram_tensor(f"eW2_{e}", [H1, H2], BF16, kind="ExternalInput")
        io[f"eb2_{e}"] = nc.dram_tensor(f"eb2_{e}", [H2], F32, kind="ExternalInput")
        io[f"eW3_{e}"] = nc.dram_tensor(f"eW3_{e}", [H2, C], BF16, kind="ExternalInput")
        io[f"eb3b_{e}"] = nc.dram_tensor(f"eb3b_{e}", [1, C], BF16, kind="ExternalInput")
    io["outbuf"] = nc.dram_tensor("outbuf", [T, CP], F32, kind="ExternalOutput")

    with tile.TileContext(nc) as tc:
        with tc.tile_pool(name="gates_pool", bufs=1) as gpool:
            acc = gpool.tile([P, NT, CP], F32, tag="acc", bufs=1)
            nc.vector.memset(acc[:], 0.0)
            pools = {"gates": gpool, "acc": acc}
            gates = _router(nc, tc, io, pools)
            _experts(nc, tc, io, pools, gates)

    nc.compile()
    return nc


def _get_built():
    global _built
    if _built is None:
        _built = build()
    return _built


def make_in_maps(inputs):
    """Shard FULL inputs into per-core in_maps."""
    x = np.asarray(inputs["x"], np.float32)
    eW1 = np.asarray(inputs["eW1"], np.float32)
    eW2 = np.asarray(inputs["eW2"], np.float32)
    eW3 = np.asarray(inputs["eW3"], np.float32)
    eb1 = np.asarray(inputs["eb1"], np.float32)
    eb2 = np.asarray(inputs["eb2"], np.float32)
    eb3 = np.asarray(inputs["eb3"], np.float32)
    shared = {}
    for k in ("rW1", "rb1", "rW2", "rb2", "rW3", "rb3"):
        shared[k] = np.ascontiguousarray(np.asarray(inputs[k], np.float32))
    for e in range(E):
        shared[f"eW1_{e}"] = np.ascontiguousarray(eW1[e].astype(ml_dtypes.bfloat16))
        shared[f"eW2_{e}"] = np.ascontiguousarray(eW2[e].astype(ml_dtypes.bfloat16))
        shared[f"eW3_{e}"] = np.ascontiguousarray(eW3[e].astype(ml_dtypes.bfloat16))
        shared[f"eb1_{e}"] = np.ascontiguousarray(eb1[e])
        shared[f"eb2_{e}"] = np.ascontiguousarray(eb2[e])
        shared[f"eb3b_{e}"] = np.ascontiguousarray(
            eb3[e].astype(ml_dtypes.bfloat16)[None, :]
        )
    in_maps = []
    for c in range(NCORES):
        xs = x[c * T : (c + 1) * T]
        xT = np.ascontiguousarray(xs.T)
        m = dict(shared)
        m["xT_f32"] = xT
        m["xT_bf16"] = xT.astype(ml_dtypes.bfloat16)
        in_maps.append(m)
    return in_maps


def kernel(**inputs):
    assert int(inputs.get("top_k", 2)) == 2
    nc = _get_built()
    in_maps = make_in_maps(inputs)
    res = run_bass_kernel_spmd(nc, in_maps, core_ids=list(range(NCORES)))
    buf = np.concatenate([res.results[c]["outbuf"] for c in range(NCORES)], axis=0)
    return np.ascontiguousarray(buf[:, :C]), np.ascontiguousarray(buf[:, C : C + E])
